# revision 1
# baseline (speedup 1.0000x reference)
"""Weighted Pearson correlation (six fused global reductions) on 8 trn2 cores.

Sharding: data-parallel over the flat N=2^25 dimension; each core reduces its
4M-element shard to a small set of partial sums which the host combines.

Per-core engine split (variant=1 default; ~125-145us/core for the 48MiB shard,
vs the ~140us HBM-stack roofline / ~116us SBUF-fabric ceiling):
  - DVE    : 3 fused affine_mul_reduce ops/tile -> products nx=n*x, ny=n*y
             plus the cancellation-sensitive sums (sum_nx, sum_ny, sum_nxy),
             all in fp32.
  - ACT    : 1 activation-Copy op/tile with accum_out -> sum_n.
  - PE     : 2 fp32 "diagonal" diag-matmul chains: diag(ny_c^T @ y_c) and
             diag(nx_c^T @ x_c) accumulated in two PSUM banks across all
             chunks/tiles -> per-column partials of sum_ny2 / sum_nx2.
  - GPSIMD : idle. Keeping it off the SBUF port it shares with DVE is worth
             ~34us/pass vs the old variant-3 split (GPSIMD mul + ACT accum
             for sum_nxx): any GpSimd op serializes against DVE's 2-port
             ops on the shared-port exclusive lock.
Host: gathers per-core partials (a few KB), reduces in float64, applies the
12-flop correlation formula.
"""

import numpy as np
from contextlib import nullcontext as _nullcontext

import concourse.bass as bass
import concourse.bacc as bacc
import concourse.tile as tile
from concourse import mybir
from concourse.bass_utils import run_bass_kernel_spmd

N_TOTAL = 33554432  # 2^25
N_CORES = 8
P = 128  # SBUF partitions

# Per-core shard: 4194304 elements = T tiles of [P, F]
F = 1024
T = N_TOTAL // N_CORES // (P * F)  # 32

_F32 = mybir.dt.float32
_MULT = mybir.AluOpType.mult
_ADD = mybir.AluOpType.add
_COPY = mybir.ActivationFunctionType.Copy


def build_nc(tiles=T, free=F, in_bufs=8, prod_bufs=2, rounds=1, variant=1,
             loop_trip=0, staggered=False):
    """Build the per-core Bass program. All 8 cores run this same program on
    their own shard (inputs shaped [tiles, 128, free])."""
    f = free
    c128 = f // 128  # stationary operand is at most 128 columns

    nc = bacc.Bacc(None)
    xs = nc.dram_tensor("xs", [tiles, P, f], _F32, kind="ExternalInput")
    ys = nc.dram_tensor("ys", [tiles, P, f], _F32, kind="ExternalInput")
    ns = nc.dram_tensor("ns", [tiles, P, f], _F32, kind="ExternalInput")
    # Partial-sum outputs: host finishes the reduction.
    # rows: 0=sum_nx, 1=sum_ny, 2=sum_nxy, 3=sum_n, 4=sum_nxx
    o_stats = nc.dram_tensor("o_stats", [6, P, tiles], _F32, kind="ExternalOutput")
    o_diag = nc.dram_tensor("o_diag", [P, P], _F32, kind="ExternalOutput")
    # variant 1: sum_nxx comes from a second PE diag pair instead of
    # GPSIMD mul + ACT accum; its diagonal lands in o_diag2.
    o_diag2 = nc.dram_tensor("o_diag2", [P, P], _F32, kind="ExternalOutput")
    # Tiny passthrough (tick->tock) so a bench harness can chain executions
    # with a data dependency; costs two 4KB DMAs.
    tick = nc.dram_tensor("tick", [P, 8], _F32, kind="ExternalInput")
    tock = nc.dram_tensor("tock", [P, 8], _F32, kind="ExternalOutput")

    with tile.TileContext(nc) as tc:
        with (
            tc.tile_pool(name="ins", bufs=in_bufs) as inp,
            tc.tile_pool(name="prods", bufs=prod_bufs) as prods,
            tc.tile_pool(name="acc", bufs=1) as accp,
            tc.tile_pool(name="psum", bufs=1, space="PSUM") as psump,
        ):
            stats_x = accp.tile([P, tiles], _F32, tag="sx")
            stats_y = accp.tile([P, tiles], _F32, tag="sy")
            stats_xy = accp.tile([P, tiles], _F32, tag="sxy")
            stats_n = accp.tile([P, tiles], _F32, tag="sn")
            stats_xx = accp.tile([P, tiles], _F32, tag="sxx")
            stats_yy2 = accp.tile([P, tiles], _F32, tag="syy2")
            if variant not in (0, 3):
                nc.vector.memset(stats_xx[:], 0.0)
            if variant != 2:
                nc.vector.memset(stats_yy2[:], 0.0)

            psum_yy = psump.tile([P, P], _F32, tag="pyy")
            psum_xx = psump.tile([P, P], _F32, tag="pxx")

            n_iter = rounds * tiles
            # loop_trip>0 wraps the pass loop in a tc.For_i hardware loop:
            # the whole multi-pass stream runs as ONE device execution
            # (bench harnesses use this for overlap-immune timing; each
            # body iteration recomputes the same correct partials).
            loop_cm = (
                tc.For_i(0, loop_trip, 1, staggered_reset=staggered)
                if loop_trip
                else _nullcontext()
            )
            with loop_cm:
                for rt in range(n_iter):
                    t = rt % tiles
                    x_t = inp.tile([P, f], _F32, tag="x")
                    y_t = inp.tile([P, f], _F32, tag="y")
                    n_t = inp.tile([P, f], _F32, tag="n")
                    if variant == 3:
                        h = f // 2
                        for src, dst in ((ys, y_t), (ns, n_t), (xs, x_t)):
                            nc.sync.dma_start(out=dst[:, 0:h], in_=src[t][:, 0:h])
                            nc.sync.dma_start(out=dst[:, h:f], in_=src[t][:, h:f])
                    else:
                        # y and n first: the first DVE op (ny) consumes them.
                        nc.sync.dma_start(out=y_t[:], in_=ys[t])
                        nc.sync.dma_start(out=n_t[:], in_=ns[t])
                        nc.sync.dma_start(out=x_t[:], in_=xs[t])

                    nx_t = prods.tile([P, f], _F32, tag="nx")
                    ny_t = prods.tile([P, f], _F32, tag="ny")
                    junk_t = prods.tile([P, f], _F32, tag="junk")
                    ajunk_t = prods.tile([P, f], _F32, tag="ajunk")
                    if variant in (0, 2, 3):
                        nxx_t = prods.tile([P, f], _F32, tag="nxx")
                        ajunk2_t = prods.tile([P, f], _F32, tag="ajunk2")

                    # DVE: products + fused free-axis sums (custom-DVE op:
                    # out = (in0*1+0)*in1, accum_out = sum(out)).
                    # ny first: it feeds the PE diag matmuls, shortening PE's
                    # per-tile idle gap (HAM re-throttle avoidance).
                    nc.vector.affine_mul_reduce(
                        out=ny_t[:], accum_out=stats_y[:, t : t + 1],
                        in0=y_t[:], in1=n_t[:], scale=1.0, bias=0.0,
                    )
                    nc.vector.affine_mul_reduce(
                        out=nx_t[:], accum_out=stats_x[:, t : t + 1],
                        in0=x_t[:], in1=n_t[:], scale=1.0, bias=0.0,
                    )
                    nc.vector.affine_mul_reduce(
                        out=junk_t[:], accum_out=stats_xy[:, t : t + 1],
                        in0=nx_t[:], in1=y_t[:], scale=1.0, bias=0.0,
                    )

                    if variant in (0, 3):
                        # GPSIMD: nxx = nx * x.
                        nc.gpsimd.tensor_tensor(
                            out=nxx_t[:], in0=nx_t[:], in1=x_t[:], op=_MULT
                        )
                        # ACT: free-axis sum of nxx via Copy + accumulate.
                        nc.scalar.activation(
                            out=ajunk2_t[:], in_=nxx_t[:], func=_COPY,
                            accum_out=stats_xx[:, t : t + 1],
                        )
                    elif variant == 1:
                        # PE: second diag pair (nx, x) -> sum n*x^2; GPSIMD idle.
                        for c in range(c128):
                            s = bass.ts(c, 128)
                            nc.tensor.matmul(
                                psum_xx[:],
                                nx_t[:, s],
                                x_t[:, s],
                                start=(rt == 0 and c == 0),
                                stop=(rt == n_iter - 1 and c == c128 - 1),
                            )
                    elif variant == 2:
                        # Like variant 0, plus: offload 1/4 of the (ny,y) diag
                        # columns from PE to GPSIMD mul + ACT accum.
                        split = f // 4
                        nc.gpsimd.tensor_tensor(
                            out=nxx_t[:], in0=nx_t[:], in1=x_t[:], op=_MULT
                        )
                        nc.scalar.activation(
                            out=ajunk2_t[:], in_=nxx_t[:], func=_COPY,
                            accum_out=stats_xx[:, t : t + 1],
                        )
                        nyy_t = prods.tile([P, f // 4], _F32, tag="nyy")
                        ajunk3_t = prods.tile([P, f // 4], _F32, tag="ajunk3")
                        nc.gpsimd.tensor_tensor(
                            out=nyy_t[:], in0=ny_t[:, 0:split], in1=y_t[:, 0:split],
                            op=_MULT,
                        )
                        nc.scalar.activation(
                            out=ajunk3_t[:], in_=nyy_t[:], func=_COPY,
                            accum_out=stats_yy2[:, t : t + 1],
                        )

                    # ACT: free-axis sum of n via Copy + accumulate.
                    nc.scalar.activation(
                        out=ajunk_t[:], in_=n_t[:], func=_COPY,
                        accum_out=stats_n[:, t : t + 1],
                    )

                    # PE: fp32 diag-matmuls -> sum over chunks of ny_chunk^T @ y_chunk;
                    # only the accumulated diagonal is meaningful (= sum n*y^2).
                    c_lo = (c128 // 4) if variant == 2 else 0
                    for c in range(c_lo, c128):
                        s = bass.ts(c, 128)
                        nc.tensor.matmul(
                            psum_yy[:],
                            ny_t[:, s],
                            y_t[:, s],
                            start=(rt == 0 and c == c_lo),
                            stop=(rt == n_iter - 1 and c == c128 - 1),
                        )

            nc.sync.dma_start(out=o_stats[0], in_=stats_x[:])
            nc.sync.dma_start(out=o_stats[1], in_=stats_y[:])
            nc.sync.dma_start(out=o_stats[2], in_=stats_xy[:])
            nc.sync.dma_start(out=o_stats[3], in_=stats_n[:])
            nc.sync.dma_start(out=o_stats[4], in_=stats_xx[:])
            nc.sync.dma_start(out=o_stats[5], in_=stats_yy2[:])
            # DMA cannot read PSUM: bounce through SBUF.
            sb_yy = accp.tile([P, P], _F32, tag="sbyy")
            nc.vector.tensor_copy(out=sb_yy[:], in_=psum_yy[:])
            nc.sync.dma_start(out=o_diag[:], in_=sb_yy[:])
            sb_xx = accp.tile([P, P], _F32, tag="sbxx2")
            if variant == 1:
                nc.vector.tensor_copy(out=sb_xx[:], in_=psum_xx[:])
            else:
                nc.vector.memset(sb_xx[:], 0.0)
            nc.sync.dma_start(out=o_diag2[:], in_=sb_xx[:])
            # tick -> tock passthrough gated on the final results (sb_yy
            # transitively depends on every input DMA + DVE op via the PSUM
            # chains; stats_n covers the ACT chain). This gives a bench
            # harness a true end-to-end data dependency for chaining
            # executions back-to-back: without it, tock's buffer can become
            # ready long before the kernel finishes and chained executions
            # overlap on the device, corrupting paired timings.
            tick_t = accp.tile([P, 8], _F32, tag="tick")
            nc.sync.dma_start(out=tick_t[:], in_=tick[:])
            tock_sb = accp.tile([P, 8], _F32, tag="tock_sb")
            nc.vector.tensor_tensor(
                out=tock_sb[:], in0=tick_t[:], in1=sb_yy[:, 0:8], op=_ADD
            )
            tock_sb2 = accp.tile([P, 8], _F32, tag="tock_sb2")
            nc.vector.tensor_tensor(
                out=tock_sb2[:], in0=tock_sb[:],
                in1=stats_n[:, tiles - 8 : tiles], op=_ADD,
            )
            nc.sync.dma_start(out=tock[:], in_=tock_sb2[:])

    nc.finalize()
    return nc


_NC_CACHE = None


def _get_nc():
    global _NC_CACHE
    if _NC_CACHE is None:
        _NC_CACHE = build_nc()
    return _NC_CACHE


def combine_partials(results):
    """Host-side all-reduce of the per-core partials + correlation formula."""
    sum_x = sum_y = sum_prod = sum_n = sum_x2 = sum_y2 = 0.0
    for r in results:
        st = np.asarray(r["o_stats"], dtype=np.float64)
        sum_x += st[0].sum()
        sum_y += st[1].sum()
        sum_prod += st[2].sum()
        sum_n += st[3].sum()
        sum_x2 += st[4].sum()
        sum_x2 += np.diag(np.asarray(r["o_diag2"], dtype=np.float64)).sum()
        sum_y2 += np.diag(np.asarray(r["o_diag"], dtype=np.float64)).sum()
        sum_y2 += st[5].sum()
    numerator = sum_n * sum_prod - sum_x * sum_y
    denominator = np.sqrt(sum_n * sum_x2 - sum_x * sum_x) * np.sqrt(
        sum_n * sum_y2 - sum_y * sum_y
    )
    return np.asarray([numerator / denominator], dtype=np.float32)


def kernel(xs, ys, ns, **run_kwargs):
    xs = np.ascontiguousarray(np.asarray(xs, dtype=np.float32)).reshape(
        N_CORES, T, P, F
    )
    ys = np.ascontiguousarray(np.asarray(ys, dtype=np.float32)).reshape(
        N_CORES, T, P, F
    )
    ns = np.ascontiguousarray(np.asarray(ns, dtype=np.float32)).reshape(
        N_CORES, T, P, F
    )
    zt = np.zeros((P, 8), dtype=np.float32)
    in_maps = [
        {"xs": xs[c], "ys": ys[c], "ns": ns[c], "tick": zt} for c in range(N_CORES)
    ]
    res = run_bass_kernel_spmd(
        _get_nc(), in_maps, core_ids=list(range(N_CORES)), **run_kwargs
    )
    return combine_partials(res.results)



# revision 2
# speedup vs baseline: 1.0151x; 1.0151x over previous
"""Weighted Pearson correlation (six fused global reductions) on 8 trn2 cores.

fp16 streaming kernel: the host casts the three fp32 input streams to fp16,
halving HBM traffic per core from 48 MiB to 24 MiB. Input rounding at ~5e-4
rms perturbs the correlation by ~1.7e-3 relative (measured vs the fp32
reference) -- far inside the 2e-2 budget -- while the six accumulations
themselves stay fp32 (PSUM / accum_out). Per-pass time goes from the
~140us fp32 HBM-stack roofline to ~70us.

Sharding: data-parallel over the flat N=2^25 dimension; each core reduces
its 4M-element shard (8 tiles of [128, 4096]) to a few KB of partials which
the host combines in float64.

Per-core engine split (rates measured on this part):
  - DVE : 2 plain fp16 tensor_tensor mults per tile (2x_1p perf mode,
          ~2.2us/tile): nx = n*x, ny = n*y. Plain TT is the only 2-src DVE
          op with a 2x uop; the reduce-bearing ops (tensor_tensor_reduce,
          affine_mul_reduce) are 1x-only and would not keep up.
  - ACT : 1 activation-Copy per tile with accum_out -> sum_n (ScalarE reads
          fp16 at 1x, ~4us/tile; only one op per tile fits the budget).
  - PE  : per 128-col chunk: stationary ny -> moving y (diag accumulates
          sum_ny2) and moving ones[128,1] (per-column sums -> sum_ny);
          stationary nx -> moving x (sum_nx2), moving y (sum_nxy), moving
          ones (sum_nx). fp16 streams 1 col/cycle (fp32 would be 4) and
          FWL weight loads overlap, ~58ns per 128-col matmul.
  - GPSIMD: idle (GpSimd ops serialize against DVE 2-port ops on the shared
          SBUF port).
All engines sit below the ~70us DMA floor, so the kernel is HBM-bound.
Host: gathers per-core partials, reduces in float64, applies the
correlation formula.
"""

import numpy as np
from contextlib import nullcontext as _nullcontext

import concourse.bass as bass
import concourse.bacc as bacc
import concourse.tile as tile
from concourse import mybir
from concourse.bass_utils import run_bass_kernel_spmd

N_TOTAL = 33554432  # 2^25
N_CORES = 8
P = 128

F = 4096
T = N_TOTAL // N_CORES // (P * F)  # 8

_F16 = mybir.dt.float16
_F32 = mybir.dt.float32
_MULT = mybir.AluOpType.mult
_ADD = mybir.AluOpType.add
_COPY = mybir.ActivationFunctionType.Copy


def build_nc(tiles=T, free=F, in_bufs=5, prod_bufs=3, junk_bufs=1,
             rounds=1, loop_trip=0, staggered=False):
    """Per-core Bass program; all 8 cores run it on their own shard
    (inputs shaped [tiles, 128, free] fp16).

    rounds>1 repeats the pass over the shard inside the body; loop_trip>0
    wraps the whole body in a tc.For_i hardware loop -- bench harnesses use
    these for single-execution paired timing (each iteration recomputes the
    same correct partials)."""
    f = free
    c128 = f // 128

    nc = bacc.Bacc(None)
    xs = nc.dram_tensor("xs", [tiles, P, f], _F16, kind="ExternalInput")
    ys = nc.dram_tensor("ys", [tiles, P, f], _F16, kind="ExternalInput")
    ns = nc.dram_tensor("ns", [tiles, P, f], _F16, kind="ExternalInput")
    # per-tile free-axis sums of n (host sums over [P, tiles])
    o_stats = nc.dram_tensor("o_stats", [P, tiles], _F32, kind="ExternalOutput")
    # diag accumulators: 0=sum_nx2, 1=sum_ny2, 2=sum_nxy (only diag matters)
    o_diag = nc.dram_tensor("o_diag", [3, P, P], _F32, kind="ExternalOutput")
    # per-stationary-column sums: 0=sum_nx, 1=sum_ny (host sums over [P])
    o_sums = nc.dram_tensor("o_sums", [2, P, 1], _F32, kind="ExternalOutput")
    # tick->tock passthrough so a bench harness can chain executions with a
    # true data dependency (prevents device-side overlap of chained calls).
    tick = nc.dram_tensor("tick", [P, 8], _F32, kind="ExternalInput")
    tock = nc.dram_tensor("tock", [P, 8], _F32, kind="ExternalOutput")

    with tile.TileContext(nc) as tc:
        with (
            tc.tile_pool(name="ins", bufs=in_bufs) as inp,
            tc.tile_pool(name="prods", bufs=prod_bufs) as prods,
            tc.tile_pool(name="junkp", bufs=junk_bufs) as junkp,
            tc.tile_pool(name="acc", bufs=1) as accp,
            tc.tile_pool(name="psum", bufs=1, space="PSUM") as psump,
        ):
            stats_n = accp.tile([P, tiles], _F32, tag="sn")

            psum_xx = psump.tile([P, P], _F32, tag="pxx")
            psum_yy = psump.tile([P, P], _F32, tag="pyy")
            psum_xy = psump.tile([P, P], _F32, tag="pxy")
            psum_snx = psump.tile([P, 2], _F32, tag="psnx")
            psum_sny = psump.tile([P, 2], _F32, tag="psny")

            ones_t = accp.tile([P, 1], _F16, tag="ones")
            nc.vector.memset(ones_t[:], 1.0)

            # pin the one-time ACT table load before the loop
            warm = accp.tile([P, 8], _F32, tag="warm")
            nc.vector.memset(warm[:], 0.0)
            nc.scalar.activation(
                out=warm[:], in_=warm[:], func=_COPY,
                accum_out=stats_n[:, 0:1],
            )

            n_iter = rounds * tiles
            loop_cm = (
                tc.For_i(0, loop_trip, 1, staggered_reset=staggered)
                if loop_trip
                else _nullcontext()
            )
            with loop_cm:
                for rt in range(n_iter):
                    t = rt % tiles
                    first = rt == 0
                    last = rt == n_iter - 1
                    x_t = inp.tile([P, f], _F16, tag="x")
                    y_t = inp.tile([P, f], _F16, tag="y")
                    n_t = inp.tile([P, f], _F16, tag="n")
                    # y and n first: the first DVE op (ny) consumes them.
                    nc.sync.dma_start(out=y_t[:], in_=ys[t])
                    nc.sync.dma_start(out=n_t[:], in_=ns[t])
                    nc.sync.dma_start(out=x_t[:], in_=xs[t])

                    ny_t = prods.tile([P, f], _F16, tag="ny")
                    nx_t = prods.tile([P, f], _F16, tag="nx")

                    # DVE: fp16 products at 2x_1p.
                    nc.vector.tensor_tensor(
                        out=ny_t[:], in0=n_t[:], in1=y_t[:], op=_MULT
                    )
                    nc.vector.tensor_tensor(
                        out=nx_t[:], in0=n_t[:], in1=x_t[:], op=_MULT
                    )

                    # ACT: free-axis sum of n via Copy + accum -> sum_n.
                    junk_t = junkp.tile([P, f], _F16, tag="junk")
                    nc.scalar.activation(
                        out=junk_t[:], in_=n_t[:], func=_COPY,
                        accum_out=stats_n[:, t : t + 1],
                    )

                    # PE: per chunk, reuse each stationary for all its movers.
                    for c in range(c128):
                        s = bass.ts(c, 128)
                        st_first = first and c == 0
                        st_last = last and c == c128 - 1
                        nc.tensor.matmul(
                            psum_yy[:], ny_t[:, s], y_t[:, s],
                            start=st_first, stop=st_last,
                        )
                        nc.tensor.matmul(
                            psum_sny[:, 0:1], ny_t[:, s], ones_t[:],
                            start=st_first, stop=st_last,
                        )
                        nc.tensor.matmul(
                            psum_xx[:], nx_t[:, s], x_t[:, s],
                            start=st_first, stop=st_last,
                        )
                        nc.tensor.matmul(
                            psum_xy[:], nx_t[:, s], y_t[:, s],
                            start=st_first, stop=st_last,
                        )
                        nc.tensor.matmul(
                            psum_snx[:, 0:1], nx_t[:, s], ones_t[:],
                            start=st_first, stop=st_last,
                        )

            nc.sync.dma_start(out=o_stats[:], in_=stats_n[:])
            # DMA cannot read PSUM: bounce through SBUF.
            sb_xx = accp.tile([P, P], _F32, tag="sbxx")
            nc.vector.tensor_copy(out=sb_xx[:], in_=psum_xx[:])
            nc.sync.dma_start(out=o_diag[0], in_=sb_xx[:])
            sb_yy = accp.tile([P, P], _F32, tag="sbyy")
            nc.vector.tensor_copy(out=sb_yy[:], in_=psum_yy[:])
            nc.sync.dma_start(out=o_diag[1], in_=sb_yy[:])
            sb_xy = accp.tile([P, P], _F32, tag="sbxy")
            nc.vector.tensor_copy(out=sb_xy[:], in_=psum_xy[:])
            nc.sync.dma_start(out=o_diag[2], in_=sb_xy[:])
            sb_sums = accp.tile([P, 2], _F32, tag="sbsums")
            nc.vector.tensor_copy(out=sb_sums[:, 0:1], in_=psum_snx[:, 0:1])
            nc.vector.tensor_copy(out=sb_sums[:, 1:2], in_=psum_sny[:, 0:1])
            nc.sync.dma_start(out=o_sums[0], in_=sb_sums[:, 0:1])
            nc.sync.dma_start(out=o_sums[1], in_=sb_sums[:, 1:2])

            # tick -> tock passthrough gated on the final results (sb_xy
            # depends transitively on every input DMA and both DVE products
            # via the PSUM chains; stats_n covers the ACT chain).
            tick_t = accp.tile([P, 8], _F32, tag="tick")
            nc.sync.dma_start(out=tick_t[:], in_=tick[:])
            tock_sb = accp.tile([P, 8], _F32, tag="tock_sb")
            nc.vector.tensor_tensor(
                out=tock_sb[:], in0=tick_t[:], in1=sb_xy[:, 0:8], op=_ADD
            )
            tock_sb2 = accp.tile([P, 8], _F32, tag="tock_sb2")
            nc.vector.tensor_tensor(
                out=tock_sb2[:], in0=tock_sb[:],
                in1=stats_n[:, tiles - 8 : tiles], op=_ADD,
            )
            nc.sync.dma_start(out=tock[:], in_=tock_sb2[:])

    nc.finalize()
    return nc


_NC_CACHE = None


def _get_nc():
    global _NC_CACHE
    if _NC_CACHE is None:
        _NC_CACHE = build_nc()
    return _NC_CACHE


def combine_partials(results):
    """Host-side all-reduce of per-core partials + correlation formula."""
    sum_n = sum_x = sum_y = sum_x2 = sum_y2 = sum_prod = 0.0
    for r in results:
        sum_n += np.asarray(r["o_stats"], dtype=np.float64).sum()
        dg = np.asarray(r["o_diag"], dtype=np.float64)
        sum_x2 += np.trace(dg[0])
        sum_y2 += np.trace(dg[1])
        sum_prod += np.trace(dg[2])
        sm = np.asarray(r["o_sums"], dtype=np.float64)
        sum_x += sm[0].sum()
        sum_y += sm[1].sum()
    numerator = sum_n * sum_prod - sum_x * sum_y
    denominator = np.sqrt(sum_n * sum_x2 - sum_x * sum_x) * np.sqrt(
        sum_n * sum_y2 - sum_y * sum_y
    )
    return np.asarray([numerator / denominator], dtype=np.float32)


def _to_f16_shards(a):
    """fp32 flat [N] -> fp16 [N_CORES, T, P, F] via jax cpu (multithreaded)."""
    import jax
    cpu = jax.devices("cpu")[0]
    with jax.default_device(cpu):
        import jax.numpy as jnp
        out = np.asarray(jnp.asarray(a).astype(jnp.float16))
    return np.ascontiguousarray(out).reshape(N_CORES, T, P, F)


def kernel(xs, ys, ns, **run_kwargs):
    xs = _to_f16_shards(np.asarray(xs, dtype=np.float32))
    ys = _to_f16_shards(np.asarray(ys, dtype=np.float32))
    ns = _to_f16_shards(np.asarray(ns, dtype=np.float32))
    zt = np.zeros((P, 8), dtype=np.float32)
    in_maps = [
        {"xs": xs[c], "ys": ys[c], "ns": ns[c], "tick": zt} for c in range(N_CORES)
    ]
    res = run_bass_kernel_spmd(
        _get_nc(), in_maps, core_ids=list(range(N_CORES)), **run_kwargs
    )
    return combine_partials(res.results)


# revision 10
# speedup vs baseline: 1.0163x; 1.0012x over previous
"""Weighted Pearson correlation (six fused global reductions) on 8 trn2 cores.

fp16 streaming kernel: the host casts the three fp32 input streams to fp16,
halving HBM traffic per core from 48 MiB to 24 MiB. Input rounding at ~5e-4
rms perturbs the correlation by ~1.7e-3 relative (measured vs the fp32
reference) -- far inside the 2e-2 budget -- while the six accumulations
themselves stay fp32 (PSUM / accum_out). Per-pass time goes from the
~140us fp32 HBM-stack roofline to ~70us.

Sharding: data-parallel over the flat N=2^25 dimension; each core reduces
its 4M-element shard (8 tiles of [128, 4096]) to a few KB of partials which
the host combines in float64.

Per-core engine split (rates measured on this part):
  - DVE : 2 plain fp16 tensor_tensor mults per tile (2x_1p perf mode,
          ~2.2us/tile): nx = n*x, ny = n*y. Plain TT is the only 2-src DVE
          op with a 2x uop; the reduce-bearing ops (tensor_tensor_reduce,
          affine_mul_reduce) are 1x-only and would not keep up.
  - ACT : 1 activation-Copy per tile with accum_out -> sum_n (ScalarE reads
          fp16 at 1x, ~4us/tile; only one op per tile fits the budget).
  - PE  : per 128-col chunk: stationary ny -> moving y (diag accumulates
          sum_ny2) and moving ones[128,1] (per-column sums -> sum_ny);
          stationary nx -> moving x (sum_nx2), moving y (sum_nxy), moving
          ones (sum_nx). fp16 streams 1 col/cycle (fp32 would be 4) and
          FWL weight loads overlap, ~58ns per 128-col matmul.
  - GPSIMD: idle (GpSimd ops serialize against DVE 2-port ops on the shared
          SBUF port).
All engines sit below the ~70us DMA floor, so the kernel is HBM-bound.
Host: gathers per-core partials, reduces in float64, applies the
correlation formula.
"""

import numpy as np
from contextlib import nullcontext as _nullcontext

import concourse.bass as bass
import concourse.bacc as bacc
import concourse.tile as tile
from concourse import mybir
from concourse.bass_utils import run_bass_kernel_spmd

N_TOTAL = 33554432  # 2^25
N_CORES = 8
P = 128

F = 4096
T = N_TOTAL // N_CORES // (P * F)  # 8

_F16 = mybir.dt.float16
_F32 = mybir.dt.float32
_MULT = mybir.AluOpType.mult
_ADD = mybir.AluOpType.add
_COPY = mybir.ActivationFunctionType.Copy


def build_nc(tiles=T, free=F, in_bufs=5, prod_bufs=3, junk_bufs=1,
             rounds=1, loop_trip=0, staggered=False, pe_sn=False):
    """Per-core Bass program; all 8 cores run it on their own shard
    (inputs shaped [tiles, 128, free] fp16).

    rounds>1 repeats the pass over the shard inside the body; loop_trip>0
    wraps the whole body in a tc.For_i hardware loop -- bench harnesses use
    these for single-execution paired timing (each iteration recomputes the
    same correct partials)."""
    f = free
    c128 = f // 128

    nc = bacc.Bacc(None)
    xs = nc.dram_tensor("xs", [tiles, P, f], _F16, kind="ExternalInput")
    ys = nc.dram_tensor("ys", [tiles, P, f], _F16, kind="ExternalInput")
    ns = nc.dram_tensor("ns", [tiles, P, f], _F16, kind="ExternalInput")
    # per-tile free-axis sums of n (host sums over [P, tiles])
    o_stats = nc.dram_tensor("o_stats", [P, tiles], _F32, kind="ExternalOutput")
    # diag accumulators: 0=sum_nx2, 1=sum_ny2, 2=sum_nxy (only diag matters)
    o_diag = nc.dram_tensor("o_diag", [3, P, P], _F32, kind="ExternalOutput")
    # per-stationary-column sums: 0=sum_nx, 1=sum_ny (host sums over [P]);
    # with pe_sn a third row carries sum_n (and o_stats stays zero).
    n_sums = 3 if pe_sn else 2
    o_sums = nc.dram_tensor("o_sums", [n_sums, P, 1], _F32, kind="ExternalOutput")
    # tick->tock passthrough so a bench harness can chain executions with a
    # true data dependency (prevents device-side overlap of chained calls).
    tick = nc.dram_tensor("tick", [P, 8], _F32, kind="ExternalInput")
    tock = nc.dram_tensor("tock", [P, 8], _F32, kind="ExternalOutput")

    with tile.TileContext(nc) as tc:
        with (
            tc.tile_pool(name="ins", bufs=in_bufs) as inp,
            tc.tile_pool(name="prods", bufs=prod_bufs) as prods,
            tc.tile_pool(name="junkp", bufs=junk_bufs) as junkp,
            tc.tile_pool(name="acc", bufs=1) as accp,
            tc.tile_pool(name="psum", bufs=1, space="PSUM") as psump,
        ):
            stats_n = accp.tile([P, tiles], _F32, tag="sn")

            psum_xx = psump.tile([P, P], _F32, tag="pxx")
            psum_yy = psump.tile([P, P], _F32, tag="pyy")
            psum_xy = psump.tile([P, P], _F32, tag="pxy")
            psum_snx = psump.tile([P, 2], _F32, tag="psnx")
            psum_sny = psump.tile([P, 2], _F32, tag="psny")
            if pe_sn:
                psum_sn = psump.tile([P, 2], _F32, tag="psn")

            ones_t = accp.tile([P, 1], _F16, tag="ones")
            nc.vector.memset(ones_t[:], 1.0)

            if pe_sn:
                nc.vector.memset(stats_n[:], 0.0)
            else:
                # pin the one-time ACT table load before the loop
                warm = accp.tile([P, 8], _F32, tag="warm")
                nc.vector.memset(warm[:], 0.0)
                nc.scalar.activation(
                    out=warm[:], in_=warm[:], func=_COPY,
                    accum_out=stats_n[:, 0:1],
                )

            n_iter = rounds * tiles
            loop_cm = (
                tc.For_i(0, loop_trip, 1, staggered_reset=staggered)
                if loop_trip
                else _nullcontext()
            )
            with loop_cm:
                for rt in range(n_iter):
                    t = rt % tiles
                    first = rt == 0
                    last = rt == n_iter - 1
                    x_t = inp.tile([P, f], _F16, tag="x")
                    y_t = inp.tile([P, f], _F16, tag="y")
                    n_t = inp.tile([P, f], _F16, tag="n")
                    # y and n first: the first DVE op (ny) consumes them.
                    nc.sync.dma_start(out=y_t[:], in_=ys[t])
                    nc.sync.dma_start(out=n_t[:], in_=ns[t])
                    nc.sync.dma_start(out=x_t[:], in_=xs[t])

                    ny_t = prods.tile([P, f], _F16, tag="ny")
                    nx_t = prods.tile([P, f], _F16, tag="nx")

                    # DVE: fp16 products at 2x_1p.
                    nc.vector.tensor_tensor(
                        out=ny_t[:], in0=n_t[:], in1=y_t[:], op=_MULT
                    )
                    nc.vector.tensor_tensor(
                        out=nx_t[:], in0=n_t[:], in1=x_t[:], op=_MULT
                    )

                    if not pe_sn:
                        # ACT: free-axis sum of n via Copy + accum -> sum_n.
                        junk_t = junkp.tile([P, f], _F16, tag="junk")
                        nc.scalar.activation(
                            out=junk_t[:], in_=n_t[:], func=_COPY,
                            accum_out=stats_n[:, t : t + 1],
                        )

                    # PE: per chunk, reuse each stationary for all its movers.
                    for c in range(c128):
                        s = bass.ts(c, 128)
                        st_first = first and c == 0
                        st_last = last and c == c128 - 1
                        nc.tensor.matmul(
                            psum_yy[:], ny_t[:, s], y_t[:, s],
                            start=st_first, stop=st_last,
                        )
                        nc.tensor.matmul(
                            psum_sny[:, 0:1], ny_t[:, s], ones_t[:],
                            start=st_first, stop=st_last,
                        )
                        nc.tensor.matmul(
                            psum_xx[:], nx_t[:, s], x_t[:, s],
                            start=st_first, stop=st_last,
                        )
                        nc.tensor.matmul(
                            psum_xy[:], nx_t[:, s], y_t[:, s],
                            start=st_first, stop=st_last,
                        )
                        nc.tensor.matmul(
                            psum_snx[:, 0:1], nx_t[:, s], ones_t[:],
                            start=st_first, stop=st_last,
                        )
                        if pe_sn:
                            # stationary n chunk, moving ones -> sum_n
                            nc.tensor.matmul(
                                psum_sn[:, 0:1], n_t[:, s], ones_t[:],
                                start=st_first, stop=st_last,
                            )

            nc.sync.dma_start(out=o_stats[:], in_=stats_n[:])
            # DMA cannot read PSUM: bounce through SBUF.
            sb_xx = accp.tile([P, P], _F32, tag="sbxx")
            nc.vector.tensor_copy(out=sb_xx[:], in_=psum_xx[:])
            nc.sync.dma_start(out=o_diag[0], in_=sb_xx[:])
            sb_yy = accp.tile([P, P], _F32, tag="sbyy")
            nc.vector.tensor_copy(out=sb_yy[:], in_=psum_yy[:])
            nc.sync.dma_start(out=o_diag[1], in_=sb_yy[:])
            sb_xy = accp.tile([P, P], _F32, tag="sbxy")
            nc.vector.tensor_copy(out=sb_xy[:], in_=psum_xy[:])
            nc.sync.dma_start(out=o_diag[2], in_=sb_xy[:])
            sb_sums = accp.tile([P, n_sums], _F32, tag="sbsums")
            nc.vector.tensor_copy(out=sb_sums[:, 0:1], in_=psum_snx[:, 0:1])
            nc.vector.tensor_copy(out=sb_sums[:, 1:2], in_=psum_sny[:, 0:1])
            nc.sync.dma_start(out=o_sums[0], in_=sb_sums[:, 0:1])
            nc.sync.dma_start(out=o_sums[1], in_=sb_sums[:, 1:2])
            if pe_sn:
                nc.vector.tensor_copy(out=sb_sums[:, 2:3], in_=psum_sn[:, 0:1])
                nc.sync.dma_start(out=o_sums[2], in_=sb_sums[:, 2:3])

            # tick -> tock passthrough gated on the final results (sb_xy
            # depends transitively on every input DMA and both DVE products
            # via the PSUM chains; stats_n covers the ACT chain).
            tick_t = accp.tile([P, 8], _F32, tag="tick")
            nc.sync.dma_start(out=tick_t[:], in_=tick[:])
            tock_sb = accp.tile([P, 8], _F32, tag="tock_sb")
            nc.vector.tensor_tensor(
                out=tock_sb[:], in0=tick_t[:], in1=sb_xy[:, 0:8], op=_ADD
            )
            tock_sb2 = accp.tile([P, 8], _F32, tag="tock_sb2")
            nc.vector.tensor_tensor(
                out=tock_sb2[:], in0=tock_sb[:],
                in1=stats_n[:, tiles - 8 : tiles], op=_ADD,
            )
            nc.sync.dma_start(out=tock[:], in_=tock_sb2[:])

    nc.finalize()
    return nc


_NC_CACHE = None


def _get_nc():
    global _NC_CACHE
    if _NC_CACHE is None:
        _NC_CACHE = build_nc()
    return _NC_CACHE


def combine_partials(results):
    """Host-side all-reduce of per-core partials + correlation formula."""
    sum_n = sum_x = sum_y = sum_x2 = sum_y2 = sum_prod = 0.0
    for r in results:
        sum_n += np.asarray(r["o_stats"], dtype=np.float64).sum()
        dg = np.asarray(r["o_diag"], dtype=np.float64)
        sum_x2 += np.trace(dg[0])
        sum_y2 += np.trace(dg[1])
        sum_prod += np.trace(dg[2])
        sm = np.asarray(r["o_sums"], dtype=np.float64)
        sum_x += sm[0].sum()
        sum_y += sm[1].sum()
        if sm.shape[0] > 2:  # pe_sn build: sum_n came from PE, o_stats is 0
            sum_n += sm[2].sum()
    numerator = sum_n * sum_prod - sum_x * sum_y
    denominator = np.sqrt(sum_n * sum_x2 - sum_x * sum_x) * np.sqrt(
        sum_n * sum_y2 - sum_y * sum_y
    )
    return np.asarray([numerator / denominator], dtype=np.float32)


def _to_f16_shards(a):
    """fp32 flat [N] -> fp16 [N_CORES, T, P, F] via jax cpu (multithreaded)."""
    import jax
    cpu = jax.devices("cpu")[0]
    with jax.default_device(cpu):
        import jax.numpy as jnp
        out = np.asarray(jnp.asarray(a).astype(jnp.float16))
    return np.ascontiguousarray(out).reshape(N_CORES, T, P, F)


def kernel(xs, ys, ns, **run_kwargs):
    xs = _to_f16_shards(np.asarray(xs, dtype=np.float32))
    ys = _to_f16_shards(np.asarray(ys, dtype=np.float32))
    ns = _to_f16_shards(np.asarray(ns, dtype=np.float32))
    zt = np.zeros((P, 8), dtype=np.float32)
    in_maps = [
        {"xs": xs[c], "ys": ys[c], "ns": ns[c], "tick": zt} for c in range(N_CORES)
    ]
    res = run_bass_kernel_spmd(
        _get_nc(), in_maps, core_ids=list(range(N_CORES)), **run_kwargs
    )
    return combine_partials(res.results)
